# revision 3
# baseline (speedup 1.0000x reference)
"""AttentionSimilarity kernel for 8 TRN2 NeuronCores.

Reference computation (B=2, S=2048, D=768, H=12, Hd=64):
    q = (x @ Wq.T + bq)  -> [B,S,H,Hd]
    k = (x @ Wk.T + bk)  -> [B,S,H,Hd]
    scores = einsum("bqhd,bkhd->bhqk", q, k) / sqrt(Hd)
    out = softmax(scores, -1).mean(axis=1)   -> [B,S,S]

Sharding: data-parallel over B (2 groups of 4 cores); within a group each
core owns a 512-row q-slice and computes all 12 heads for that slice.
k-projection is replicated inside the group.  fp16 on-device (fp32 PSUM
accumulation), fp16 out (host casts to fp32).

Per-core schedule (ACT-bound, ~2.0us per step, 48 steps):
  - PSUM: two persistent [128,2048] fp32 score tiles (4 banks each = all
    8 banks), alternating per step.  One step = one (head, q-tile): four
    K=64 matmuls fill the tile's quarters, then ONE exp over all 2048
    cols with fused row-sum (accum_out) -> softmax denominator.
  - Projections (transposed layout, 6 accumulating matmuls per group,
    30 groups) borrow quarter 3 of the score tile that is idle between
    its exp and its next refill; a DVE bias-add reads the quarter out to
    SBUF while the next step's rc0-2 score matmuls provide cover.
  - DVE per step: reciprocal of the denominator + ONE fused
    scalar_tensor_tensor: acc = E*r12 + acc (f16, 2x mode).  GpSimd does
    the tiny r12 = r/12 mul.  Last head per q-tile writes the f16 output
    tile, DMA'd out on the SP queue (keeps the ACT sequencer clean).
  - Keeping PE dense (scores + interleaved projections) holds the
    2.4GHz p-state; sparse PE drops to 1.2GHz and halves matmul rate.
"""

import numpy as np

B = 2
S = 2048
D = 768
H = 12
HD = 64
P = 128
DI = D // P            # 6 chunks of the contraction dim
NCORES = 8
QPC = S // 4           # 512 q rows per core
QT = QPC // P          # 4 q-tiles per core
NCHUNK = DI            # 6 feature chunks == 6 head pairs

_BUILT = None


def _build():
    global _BUILT
    if _BUILT is not None:
        return _BUILT

    import concourse.bass as bass  # noqa: F401
    import concourse.mybir as mybir
    import concourse.tile as tile
    from concourse import bacc

    f32 = mybir.dt.float32
    f16 = mybir.dt.float16
    Alu = mybir.AluOpType
    Act = mybir.ActivationFunctionType

    nc = bacc.Bacc("TRN2", target_bir_lowering=False, debug=False,
                   num_devices=NCORES)

    xT = nc.dram_tensor("xT", [D, S], f16, kind="ExternalInput").ap()
    xqT = nc.dram_tensor("xqT", [D, QPC], f16, kind="ExternalInput").ap()
    wqT = nc.dram_tensor("wqT", [D, D], f16, kind="ExternalInput").ap()
    wkT = nc.dram_tensor("wkT", [D, D], f16, kind="ExternalInput").ap()
    bq = nc.dram_tensor("bq", [D], f32, kind="ExternalInput").ap()
    bk = nc.dram_tensor("bk", [D], f32, kind="ExternalInput").ap()
    out = nc.dram_tensor("out", [QPC, S], f16, kind="ExternalOutput").ap()

    xT_r = xT.rearrange("(c p) s -> p c s", p=P)
    xqT_r = xqT.rearrange("(c p) s -> p c s", p=P)
    wqT_r = wqT.rearrange("(c p) d -> p c d", p=P)
    wkT_r = wkT.rearrange("(c p) d -> p c d", p=P)
    bq_r = bq.rearrange("(c p) -> p c", p=P)
    bk_r = bk.rearrange("(c p) -> p c", p=P)

    with tile.TileContext(nc) as tc:
        import contextlib
        with contextlib.ExitStack() as ctx:
            consts = ctx.enter_context(tc.tile_pool(name="consts", bufs=1))
            psp = ctx.enter_context(
                tc.tile_pool(name="psp", bufs=1, space="PSUM"))
            epool = ctx.enter_context(tc.tile_pool(name="epool", bufs=3))
            dpool = ctx.enter_context(tc.tile_pool(name="dpool", bufs=6))
            outp = ctx.enter_context(tc.tile_pool(name="outp", bufs=2))

            # ---- persistent SBUF tensors ----
            xT_sb = consts.tile([P, DI, S], f16, tag="xT")
            xq_sb = consts.tile([P, DI, QPC], f16, tag="xq")
            wq_sb = consts.tile([P, DI, D], f16, tag="wq")
            wk_sb = consts.tile([P, DI, D], f16, tag="wk")
            bq_sb = consts.tile([P, DI], f32, tag="bq")
            bk_sb = consts.tile([P, DI], f32, tag="bk")
            kT_sb = consts.tile([P, DI, S], f16, tag="kT")
            qT_sb = consts.tile([P, DI, QPC], f16, tag="qT")
            accs = [consts.tile([P, S], f16, tag=f"acc{qt}",
                                name=f"acc{qt}")
                    for qt in range(QT)]

            # two persistent [128,2048] fp32 score tiles = all 8 PSUM banks
            ps_tiles = [psp.tile([P, S], f32, tag="psA", name="psA"),
                        psp.tile([P, S], f32, tag="psB", name="psB")]

            # ---- DMAs, ordered by first use; split across SP + ACT
            # hwdge queues (both idle during the ramp) ----
            # Critical path: wq/wk chunk-0 cols + xq + full xT.
            for di in range(DI):
                nc.sync.dma_start(out=wq_sb[:, di, 0:P],
                                  in_=wqT_r[:, di, 0:P])
            for di in range(DI):
                nc.scalar.dma_start(out=wk_sb[:, di, 0:P],
                                    in_=wkT_r[:, di, 0:P])
            for di in range(DI):
                nc.sync.dma_start(out=xq_sb[:, di, :], in_=xqT_r[:, di, :])
            nc.scalar.dma_start(out=bq_sb, in_=bq_r)
            nc.scalar.dma_start(out=bk_sb, in_=bk_r)
            for rc in range(4):
                eng = nc.sync if rc % 2 == 0 else nc.scalar
                rs = slice(rc * 512, (rc + 1) * 512)
                for di in range(DI):
                    eng.dma_start(out=xT_sb[:, di, rs], in_=xT_r[:, di, rs])
            for di in range(DI):
                nc.scalar.dma_start(out=wq_sb[:, di, P:D],
                                    in_=wqT_r[:, di, P:D])
                nc.sync.dma_start(out=wk_sb[:, di, P:D],
                                  in_=wkT_r[:, di, P:D])

            def proj_group(t, g, ps_q):
                """Project group g of chunk t into PSUM quarter ps_q, then
                bias-add it out to SBUF (DVE).  g==0: q rows; g==1..4:
                k range (g-1)."""
                tsl = slice(t * P, (t + 1) * P)
                if g == 0:
                    for di in range(DI):
                        nc.tensor.matmul(ps_q, wq_sb[:, di, tsl],
                                         xq_sb[:, di, :],
                                         start=(di == 0), stop=(di == DI - 1))
                    nc.vector.tensor_scalar_add(
                        out=qT_sb[:, t, :], in0=ps_q,
                        scalar1=bq_sb[:, t:t + 1])
                else:
                    rc = g - 1
                    rs = slice(rc * 512, (rc + 1) * 512)
                    for di in range(DI):
                        nc.tensor.matmul(ps_q, wk_sb[:, di, tsl],
                                         xT_sb[:, di, rs],
                                         start=(di == 0), stop=(di == DI - 1))
                    nc.vector.tensor_scalar_add(
                        out=kT_sb[:, t, rs], in0=ps_q,
                        scalar1=bk_sb[:, t:t + 1])

            # prologue: chunk-0 projections spread over psA q0-q3 + psB q0
            for g in range(5):
                ps = ps_tiles[0] if g < 4 else ps_tiles[1]
                q = g % 4 if g < 4 else 0
                proj_group(0, g, ps[:, q * 512:(q + 1) * 512])

            for s in range(H * QT):  # 48 steps: (chunk t, qt, head parity i)
                t, r = divmod(s, 8)
                qt, i = divmod(r, 2)
                h = 2 * t + i
                ps = ps_tiles[s % 2]
                ps_other = ps_tiles[1 - s % 2]
                qsl = slice(qt * P, (qt + 1) * P)
                po = i * HD

                # score matmuls: 4 quarters of the [128,2048] tile
                for rc in range(4):
                    rs = slice(rc * 512, (rc + 1) * 512)
                    nc.tensor.matmul(
                        ps[:, rs],
                        qT_sb[po:po + HD, t, qsl],
                        kT_sb[po:po + HD, t, rs],
                        start=True, stop=True)

                # next chunk's projections ride in the other tile's q3,
                # which just became free after its exp
                if t + 1 < NCHUNK and r < 5:
                    proj_group(t + 1, r, ps_other[:, 1536:2048])

                # ONE exp over the full row with fused denominator
                E = epool.tile([P, S], f16, tag="E", name=f"E_{s}")
                dden = dpool.tile([P, 1], f32, tag="d", name=f"d_{s}")
                nc.scalar.activation(out=E, in_=ps, func=Act.Exp,
                                     scale=0.125, accum_out=dden)

                r_ = dpool.tile([P, 1], f32, tag="r")
                nc.vector.reciprocal(out=r_, in_=dden)
                r12 = dpool.tile([P, 1], f32, tag="r12")
                nc.gpsimd.tensor_scalar_mul(out=r12, in0=r_,
                                            scalar1=1.0 / 12.0)

                acc = accs[qt]
                if h == 0:
                    nc.vector.tensor_scalar_mul(out=acc, in0=E, scalar1=r12)
                elif h == H - 1:
                    ot = outp.tile([P, S], f16, tag="ot")
                    nc.vector.scalar_tensor_tensor(
                        out=ot, in0=E, scalar=r12, in1=acc,
                        op0=Alu.mult, op1=Alu.add)
                    nc.sync.dma_start(out=out[qsl, :], in_=ot)
                else:
                    nc.vector.scalar_tensor_tensor(
                        out=acc, in0=E, scalar=r12, in1=acc,
                        op0=Alu.mult, op1=Alu.add)

    nc.compile()
    _BUILT = (nc,)
    return _BUILT


def make_in_maps(x, Wq, bq, Wk, bk):
    f16 = np.float16
    x = np.asarray(x, dtype=np.float32)
    wqT = np.ascontiguousarray(np.asarray(Wq, np.float32).T).astype(f16)
    wkT = np.ascontiguousarray(np.asarray(Wk, np.float32).T).astype(f16)
    bq = np.asarray(bq, np.float32)
    bk = np.asarray(bk, np.float32)
    in_maps = []
    for c in range(NCORES):
        b, qc = c // 4, c % 4
        xTb = np.ascontiguousarray(x[b].T).astype(f16)      # [768, 2048]
        xqTc = np.ascontiguousarray(xTb[:, qc * QPC:(qc + 1) * QPC])
        in_maps.append({
            "xT": xTb,
            "xqT": xqTc,
            "wqT": wqT,
            "wkT": wkT,
            "bq": bq,
            "bk": bk,
        })
    return in_maps


def run(x, Wq, bq, Wk, bk, trace=False, **trace_kwargs):
    from concourse.bass_utils import run_bass_kernel_spmd
    (nc,) = _build()
    in_maps = make_in_maps(x, Wq, bq, Wk, bk)
    res = run_bass_kernel_spmd(
        nc, in_maps, core_ids=list(range(NCORES)), trace=trace,
        **trace_kwargs)
    outp = np.zeros((B, S, S), np.float32)
    for c in range(NCORES):
        b, qc = c // 4, c % 4
        outp[b, qc * QPC:(qc + 1) * QPC, :] = \
            res.results[c]["out"].astype(np.float32)
    return outp, res


def kernel(x, Wq, bq, Wk, bk):
    outp, _ = run(x, Wq, bq, Wk, bk, trace=False)
    return outp
